# revision 47
# baseline (speedup 1.0000x reference)
"""Trainium2 Bass kernel for nn_KSpaceLoss: exact type-2 NUFFT k-space loss.

loss = 0.1 * (sum|d| / sum|a|) + 0.1 * sqrt(sum d^2 / sum a^2)
  d = (E @ x) * mask - kdata * mask,  a = kdata * mask
  E[k, n] = exp(-2j*pi * traj[:, k] . r[:, n])   (K=8192, N=96*96)

Strategy:
  * K axis: masked-out samples contribute 0 to both sums -> gather kept
    columns on host, pad to KP = 8*KL, shard over 8 cores (KL each).
  * Phase factorization: n=(nx,ny), nx=32*nx1+nx0, ny=48*ny1+ny0 gives
    E[n,k] = U[g,k] * V[m,k] with g=(nx1,ny1) in 6 groups and
    m=(nx0,ny0) in 1536 members. V ([1536,K] twiddle table, ~16% of E)
    and U ([6,K]) are host-precomputed; V is shipped as fp8e4.
  * ksp[k,c] = sum_g U[g,k] * W_g[k,c],  W_g = V^T @ x_g  as fp8e4
    DoubleRow matmuls (2 m-chunks per matmul, 256-deep contraction per
    column, 2x bf16 MAC throughput). Weights pack
    both groups of a pair into 128 columns [xr_e|xi_e|xr_o|xi_o]: stream
    Vr against that, Vi against [-xi_e|xr_e|-xi_o|xr_o], accumulating
    [Wre_e|Wim_e|Wre_o|Wim_o] in PSUM over all 12 member-chunks.
  * U applied per pair with two elementwise products (W * U-pack -> bf16)
    on DVE; each product is folded straight into kr/ki PSUM accumulators
    with +-1 sign matmuls on the PE. Pair scheduling: pairs 0-1 run
    chunk-outer through the stream-DMA arrival window; pair 2's matmuls
    run last so they hide pairs 0-1's products and folds. The k-space
    mask value and zero-padding of the gathered K axis are folded into
    the U-packs (U=0 at pads -> ksp=0, kdata=0 -> d=0).
  * Residual: combined [64,KL] d = [kr;ki] - [kdr;kdi], one fused
    square+reduce (sum d^2), a ones-fold matmul for dr^2+di^2, and one
    Sqrt activation with accumulation (sum |d|). |a| sums are computed
    on host (O(K), input-only). Host does the final scalar combine.
"""

import math

import numpy as np

import concourse.bacc as bacc
import concourse.tile as tile
from concourse import mybir
from concourse.bass_utils import run_bass_kernel_spmd

X, Y = 96, 96
C, T = 8, 4
K = 8192
N = X * Y
NCORES = 8
CST = C * T               # 32
G = 6                     # groups: nx1 in [0,3), ny1 in [0,2)
M = 1536                  # members: nx0 in [0,32), ny0 in [0,48)
MCH = M // 128            # 12 member chunks
CP = MCH // 2             # 6 DoubleRow chunk-pairs
NPAIR = G // 2            # 3 group pairs
W1, W2 = 0.1, 0.1

F32 = mybir.dt.float32
F16 = mybir.dt.float16
BF16 = mybir.dt.bfloat16
F8 = mybir.dt.float8e4
F32R = mybir.dt.float32r

KL_PRIMARY = 640          # per-core columns; covers mask count <= 5120
KL_FULL = 1024            # fallback: all 8192 columns fit


def _bank_slices(kl):
    out, j = [], 0
    while j < kl:
        je = min(j + 512, kl)
        out.append((j, je))
        j = je
    return out


def build_kernel(KL):
    nc = bacc.Bacc("TRN2", target_bir_lowering=False, debug=False,
                   num_devices=NCORES)

    w1_d = nc.dram_tensor("w1", [NPAIR, 128, CP, 2, 2, 64], F8,
                          kind="ExternalInput").ap()
    vr_d = nc.dram_tensor("vr", [CP, 128, 2, KL], F8, kind="ExternalInput").ap()
    vi_d = nc.dram_tensor("vi", [CP, 128, 2, KL], F8, kind="ExternalInput").ap()
    uc_d = nc.dram_tensor("uc", [NPAIR, 128, 2, KL], F8, kind="ExternalInput").ap()
    kdri_d = nc.dram_tensor("kdri", [2 * CST, KL], F16, kind="ExternalInput").ap()
    sgn_d = nc.dram_tensor("sgn", [128, 3, CST], BF16, kind="ExternalInput").ap()
    parts_d = nc.dram_tensor("parts", [2 * CST, 4], F32, kind="ExternalOutput").ap()

    Sqrt = mybir.ActivationFunctionType.Sqrt
    Alu = mybir.AluOpType
    DR = mybir.MatmulPerfMode.DoubleRow
    JS = _bank_slices(KL)

    def dma4(dst, src):
        # one whole-tile transfer per tensor: partition-sliced same-tile
        # DMAs serialize in the tile framework and regress badly
        nc.sync.dma_start(dst[:], src[:])

    with tile.TileContext(nc) as tc:
        with (
            tc.tile_pool(name="const", bufs=1) as cpool,
            tc.tile_pool(name="wacc", bufs=3, space="PSUM") as wpool,
            tc.tile_pool(name="fin", bufs=1, space="PSUM") as fpool,
            tc.tile_pool(name="prod", bufs=3) as prp,
            tc.tile_pool(name="resid", bufs=1) as rsp,
        ):
            # ---- constant loads, priority-ordered and queue-spread ----
            w1t = [cpool.tile([128, CP, 2, 2, 64], F8, tag=f"w1_{p}", name=f"w1t{p}")
                   for p in range(NPAIR)]
            w2t = [cpool.tile([128, CP, 2, 2, 64], F8, tag=f"w2_{p}", name=f"w2t{p}")
                   for p in range(NPAIR)]
            vr8 = [cpool.tile([128, 2, KL], F8, tag=f"vr8_{cp}", name=f"vr8{cp}")
                   for cp in range(CP)]
            vi8 = [cpool.tile([128, 2, KL], F8, tag=f"vi8_{cp}", name=f"vi8{cp}")
                   for cp in range(CP)]
            uct = [cpool.tile([128, 2, KL], F8, tag=f"uc_{p}", name=f"uct{p}")
                   for p in range(NPAIR)]
            sgn = cpool.tile([128, 3, CST], BF16, tag="sgn")
            kdri = cpool.tile([2 * CST, KL], F16, tag="kdri")

            # issue order ~= consumer deadline order
            dma4(w1t[0][:], w1_d[0])
            dma4(vr8[0][:], vr_d[0])
            dma4(vi8[0][:], vi_d[0])
            dma4(w1t[1][:], w1_d[1])
            dma4(w1t[2][:], w1_d[2])
            dma4(vr8[1][:], vr_d[1])
            dma4(vi8[1][:], vi_d[1])
            dma4(vr8[2][:], vr_d[2])
            dma4(vi8[2][:], vi_d[2])
            dma4(uct[0][:], uc_d[0])
            nc.sync.dma_start(sgn[:], sgn_d[:])
            dma4(vr8[3][:], vr_d[3])
            dma4(vi8[3][:], vi_d[3])
            dma4(uct[1][:], uc_d[1])
            dma4(vr8[4][:], vr_d[4])
            dma4(vi8[4][:], vi_d[4])
            nc.sync.dma_start(kdri[:], kdri_d[:])
            dma4(vr8[5][:], vr_d[5])
            dma4(vi8[5][:], vi_d[5])
            dma4(uct[2][:], uc_d[2])

            # derive w2 = [-xi_e|xr_e|-xi_o|xr_o] from w1 = [xr_e|xi_e|...]
            U8 = mybir.dt.uint8
            for p in range(NPAIR):
                eng = nc.vector
                eng.tensor_scalar(w2t[p][:, :, :, :, 0:32].bitcast(U8),
                                  w1t[p][:, :, :, :, 32:64].bitcast(U8),
                                  0x80, None, op0=Alu.bitwise_xor)
                eng.tensor_scalar(w2t[p][:, :, :, :, 32:64].bitcast(U8),
                                  w1t[p][:, :, :, :, 0:32].bitcast(U8),
                                  0x00, None, op0=Alu.bitwise_xor)

            parts = rsp.tile([2 * CST, 4], F32, tag="parts")
            nc.vector.memset(parts[:], 0.0)


            # ---- per pair: PSUM W accumulation -> U products -> folds ----
            kri = fpool.tile([2 * CST, 1024], F32, tag="kri")
            prods = []          # (p1, p2) awaiting fold
            fold_p = [0]

            def emit_folds():
                p = fold_p[0]
                p1, p2 = prods[p]
                for (js, je) in JS:
                    nc.tensor.matmul(kri[0:CST, js:je], sgn[:, 0, :],
                                     p1[:, js:je],
                                     start=(p == 0), stop=(p == NPAIR - 1))
                for (js, je) in JS:
                    nc.tensor.matmul(kri[CST:2 * CST, js:je], sgn[:, 1, :],
                                     p2[:, js:je],
                                     start=(p == 0), stop=(p == NPAIR - 1))
                fold_p[0] += 1

            Ws = [wpool.tile([128, 1024], F32, tag="W", name=f"W{p}")
                  for p in range(NPAIR)]

            def emit_pair_mms(p, cp):
                for (js, je) in JS:
                    nc.tensor.matmul(Ws[p][:, js:je], w1t[p][:, cp],
                                     vr8[cp][:, :, js:je],
                                     perf_mode=DR,
                                     start=(cp == 0), stop=False)
                for (js, je) in JS:
                    nc.tensor.matmul(Ws[p][:, js:je], w2t[p][:, cp],
                                     vi8[cp][:, :, js:je],
                                     perf_mode=DR,
                                     start=False, stop=(cp == CP - 1))

            def emit_products(p):
                p1 = prp.tile([128, KL], BF16, tag="p1")
                p2 = prp.tile([128, KL], BF16, tag="p2")
                nc.vector.tensor_tensor(p1[:], Ws[p][:, :KL], uct[p][:, 0],
                                        op=Alu.mult)
                nc.vector.tensor_tensor(p2[:], Ws[p][:, :KL], uct[p][:, 1],
                                        op=Alu.mult)
                prods.append((p1, p2))

            # pairs 0-1 chunk-outer through the stream-arrival window;
            # pair 2 runs last on the PE, hiding pairs 0-1's products+folds
            for cp in range(CP):
                for p in (0, 1):
                    emit_pair_mms(p, cp)
            emit_products(0)
            emit_products(1)
            for cp in range(CP):
                emit_pair_mms(2, cp)
            emit_folds()
            emit_folds()
            emit_products(2)
            emit_folds()

            # ---- residual: d = [kr;ki] - [kdr;kdi], sums ----
            d = rsp.tile([2 * CST, KL], F32, tag="d")
            sqb = rsp.tile([2 * CST, KL], BF16, tag="sqb")
            ssum = wpool.tile([128, 1024], F32, tag="W")
            t2 = rsp.tile([CST, KL], F32, tag="t2")
            # per k-slice partial sums; the js0/js1 merge happens on host
            for si, (js, je) in enumerate(JS):
                nc.vector.tensor_tensor(d[:, js:je], kri[:, js:je],
                                        kdri[:, js:je], op=Alu.subtract)
                nc.vector.scalar_tensor_tensor(
                    sqb[:, js:je], d[:, js:je], 0.0, d[:, js:je],
                    op0=Alu.bypass, op1=Alu.mult,
                    accum_out=parts[:, 2 * si + 1:2 * si + 2])
                nc.tensor.matmul(ssum[0:CST, js:je], sgn[0:2 * CST, 2, :],
                                 sqb[:, js:je], start=True, stop=True)
                nc.scalar.activation(
                    t2[:, js:je], ssum[0:CST, js:je], Sqrt,
                    accum_out=parts[0:CST, 2 * si:2 * si + 1])

            nc.sync.dma_start(parts_d[:], parts[:])

    nc.compile()
    return nc


_NC_CACHE = {}


def _get_nc(kl):
    if kl not in _NC_CACHE:
        _NC_CACHE[kl] = build_kernel(kl)
    return _NC_CACHE[kl]


def _prep_weights(images_reconstructed, sensitivity_maps):
    f8 = mybir.dt.np(F8)
    img = np.asarray(images_reconstructed)
    smaps = np.asarray(sensitivity_maps)
    x = 0.5 * img[None, ...] * smaps[..., None, None]       # (C,X,Y,1,1,T)
    xw = x.reshape(C, N, T).transpose(1, 0, 2).reshape(N, CST)  # n = nx*96+ny
    # regroup: [nx1, nx0, ny1, ny0] -> [g=(nx1,ny1), m=(nx0,ny0)]
    xg = xw.reshape(3, 32, 2, 48, CST).transpose(0, 2, 1, 3, 4).reshape(G, M, CST)
    xr = xg.real.astype(np.float32)
    xi = xg.imag.astype(np.float32)
    # w[pair, m0, cp, i, gi, :]: DoubleRow weights, m = 128*(2*cp+i) + m0;
    # block gi packs group 2p+gi of the pair as [xr|xi]
    w1 = np.empty((NPAIR, 128, CP, 2, 2, 64), np.float32)
    for p in range(NPAIR):
        for gi, g in enumerate((2 * p, 2 * p + 1)):
            for ch in range(MCH):
                cp, half = divmod(ch, 2)
                sl = slice(128 * ch, 128 * (ch + 1))
                w1[p, :, cp, half, gi, 0:32] = xr[g, sl]
                w1[p, :, cp, half, gi, 32:64] = xi[g, sl]
    return np.ascontiguousarray(w1.astype(f8))


def make_in_maps(images_reconstructed, kspace_trajectory, kspace_data,
                 kspace_mask, sensitivity_maps, KL):
    f8 = mybir.dt.np(F8)
    KP = KL * NCORES
    traj = np.asarray(kspace_trajectory).astype(np.float32)
    kdata = np.asarray(kspace_data)
    mask = np.asarray(kspace_mask).astype(np.float32).reshape(K)

    w1 = _prep_weights(images_reconstructed, sensitivity_maps)

    # gather kept columns, zero-pad to KP
    idx = np.flatnonzero(mask > 0)
    cnt = idx.size
    assert cnt <= KP, f"mask count {cnt} exceeds padded K {KP}"
    txg = np.zeros(KP, np.float64)
    tyg = np.zeros(KP, np.float64)
    txg[:cnt] = traj[0][idx]
    tyg[:cnt] = traj[1][idx]

    # V twiddle table (host, fp64 phase -> fp8): m = nx0*48 + ny0
    mm = np.arange(M)
    vx = (mm // 48 - 48).astype(np.float64)
    vy = (mm % 48 - 48).astype(np.float64)
    phs_v = vx[:, None] * txg[None, :] + vy[:, None] * tyg[None, :]  # (M, KP)
    vrf = np.cos(2 * np.pi * phs_v).astype(np.float32).astype(f8)
    vif = (-np.sin(2 * np.pi * phs_v)).astype(np.float32).astype(f8)
    # device layout [CP, 128, 2, KL-slice]; member chunk = 2*cp + i
    vr = vrf.reshape(CP, 2, 128, KP).transpose(0, 2, 1, 3)
    vi = vif.reshape(CP, 2, 128, KP).transpose(0, 2, 1, 3)

    # U twiddles with keep-mask, replicated fp8 packs
    g_idx = np.arange(G)
    phs_u = ((32 * (g_idx // 2))[:, None] * txg[None, :]
             + (48 * (g_idx % 2))[:, None] * tyg[None, :])
    ur = np.cos(2 * np.pi * phs_u)
    ui = -np.sin(2 * np.pi * phs_u)
    # fold the mask value into U (scales ksp) -- exact for any mask value
    keep = np.zeros(KP, np.float64)
    keep[:cnt] = mask[idx]
    ur *= keep[None, :]
    ui *= keep[None, :]
    uc = np.empty((NPAIR, 128, 2, KP), f8)
    for p in range(NPAIR):
        uc[p, 0:32, 0] = ur[2 * p]
        uc[p, 32:64, 0] = ui[2 * p]
        uc[p, 64:96, 0] = ur[2 * p + 1]
        uc[p, 96:128, 0] = ui[2 * p + 1]
        uc[p, 0:32, 1] = ui[2 * p]
        uc[p, 32:64, 1] = ur[2 * p]
        uc[p, 64:96, 1] = ui[2 * p + 1]
        uc[p, 96:128, 1] = ur[2 * p + 1]

    # sign matrices: fold the 4 blocks of P1/P2 (kr needs +,-,+,-; ki all +)
    # and the ones-fold pairing dr^2+di^2 (col 2)
    sgn = np.zeros((128, 3, CST), np.float32)
    for j in range(4):
        s = 1.0 if j % 2 == 0 else -1.0
        for c in range(CST):
            sgn[32 * j + c, 0, c] = s
            sgn[32 * j + c, 1, c] = 1.0
    for j in range(2):
        for c in range(CST):
            sgn[32 * j + c, 2, c] = 1.0
    sgn = sgn.astype(mybir.dt.np(BF16))

    # kdata at kept columns (mask=1 there); (K, CST) with c = coil*T + t
    kdm = kdata.reshape(C, K, T).transpose(1, 0, 2).reshape(K, CST)
    kdm = kdm * mask[:, None]
    kg = np.zeros((KP, CST), np.complex64)
    kg[:cnt] = kdm[idx]

    in_maps = []
    for i in range(NCORES):
        ksl = slice(i * KL, (i + 1) * KL)
        kdri = np.concatenate([kg.real[ksl].T, kg.imag[ksl].T], axis=0)
        in_maps.append({
            "w1": w1,
            "vr": np.ascontiguousarray(vr[:, :, :, ksl]),
            "vi": np.ascontiguousarray(vi[:, :, :, ksl]),
            "uc": np.ascontiguousarray(uc[:, :, :, ksl]),
            "kdri": np.ascontiguousarray(kdri.astype(np.float16)),
            "sgn": sgn,
        })

    # host |a| sums (input-only, O(K))
    am = np.abs(kdm[idx]).astype(np.float64)
    sa1 = am.sum()
    sa2 = (am * am).sum()
    return in_maps, sa1, sa2


def combine(parts_list, sa1, sa2):
    tot0 = 0.0
    tot1 = 0.0
    for p in parts_list:
        p = p.astype(np.float64)
        tot0 += p[0:CST, 0].sum() + p[0:CST, 2].sum()
        tot1 += p[:, 1].sum() + p[:, 3].sum()
    loss = W1 * (tot0 / sa1) + W2 * math.sqrt(tot1 / sa2)
    return np.asarray(loss, dtype=np.float32)


def kernel(images_reconstructed, kspace_trajectory, kspace_data,
           kspace_mask, sensitivity_maps, _trace=False):
    mask = np.asarray(kspace_mask).astype(np.float32).reshape(K)
    cnt = int((mask > 0).sum())
    KL = KL_PRIMARY if cnt <= KL_PRIMARY * NCORES else KL_FULL
    nc = _get_nc(KL)
    in_maps, sa1, sa2 = make_in_maps(images_reconstructed, kspace_trajectory,
                                     kspace_data, kspace_mask,
                                     sensitivity_maps, KL)
    res = run_bass_kernel_spmd(nc, in_maps, core_ids=list(range(NCORES)),
                               trace=_trace)
    out = combine([res.results[i]["parts"] for i in range(NCORES)], sa1, sa2)
    if _trace:
        return out, res
    return out

